# revision 6
# baseline (speedup 1.0000x reference)
"""Trainium2 Bass kernel for nn_DecoderLayer_50534585205086 (v2, all-bf16).

Sharding: 8 cores = 4 batches x 2 sequence halves; T=2048 tokens/core.
Pipeline per core: embed add, QKV proj, causal dwconv, avg-pool (bf16
matmul operands); pairwise AllGather of pooled K (and V with a baked-in
ones row for the softmax denominator); pooled causal attention with
V transposed to token-major via XBAR DMA; wup+wc folded on host into one
wc2 GEMM applied on POOLED tokens (attention output is constant within
each 4-token group), LN1 also on pooled tokens, upsample x4 only at the
residual add via broadcast APs; FFN with w1 streamed + w2 resident in
bf16; LN2 + residual; PE transpose to token-major f32 output.
"""

import numpy as np
import ml_dtypes
from contextlib import ExitStack

import concourse.bass as bass
import concourse.tile as tile
from concourse import bacc, mybir
from concourse.bass import ts
from concourse.bass_utils import run_bass_kernel_spmd
from concourse.masks import make_identity

F32 = mybir.dt.float32
BF16 = mybir.dt.bfloat16
AL = mybir.AluOpType
AF = mybir.ActivationFunctionType

N_CORES = 8
B, S_FULL, DM, H, DD, DF = 4, 4096, 1024, 16, 64, 4096
KER, KW = 4, 3
NORM = float(DD) ** -0.25
EPS = 1e-6
CT = DM // 128   # 8 channel tiles
FT = DF // 128   # 32 ffn tiles
HALO = 8
VP = 72          # v rows per head incl ones row + pad to 16 multiple

# packed constant-vector column offsets: name -> (offset, width)
_COFF = {}
_off = 0
for _nm, _w in [("gq", CT * 6), ("gk", CT * 6), ("gv", CT * 6),
                ("dbq", CT), ("dbk", CT), ("dbv", CT),
                ("bq", CT), ("bk", CT), ("bv", CT), ("bc2", CT), ("b2", CT),
                ("g1", CT), ("be1", CT), ("g2", CT), ("be2", CT),
                ("b1", FT), ("hmask", HALO), ("hcorr", 1)]:
    _COFF[_nm] = (_off, _w)
    _off += _w
NCONST = _off


def _chunks(total, width):
    out = []
    c0 = 0
    while c0 < total:
        w = min(width, total - c0)
        out.append((c0, w))
        c0 += w
    return out


def build_program(S=S_FULL, mock_collective=False):
    T = S // 2           # tokens per core
    TH = T + HALO
    L = S // KER         # pooled length per batch
    LLOC = L // 2        # pooled positions owned per core
    MT = L // 128        # m tiles (keys)

    nc = bacc.Bacc("TRN2", target_bir_lowering=False, debug=False,
                   num_devices=N_CORES)

    def din(name, shape, dt=BF16):
        return nc.dram_tensor(name, list(shape), dt, kind="ExternalInput").ap()

    xe_ap = din("xe", [DM, TH])
    xp_ap = din("xp", [DM, TH])
    wq_ap = din("wq", [128, CT, DM])
    wk_ap = din("wk", [128, CT, DM])
    wv_ap = din("wv", [128, CT, DM])
    wc2_ap = din("wc2", [CT, 128, DM])
    w1_ap = din("w1", [128, FT, DM])
    w2_ap = din("w2", [FT, 128, DM])
    cvec_ap = din("cvec", [128, NCONST], F32)
    mask_ap = din("mask", [L, LLOC])

    y_ap = nc.dram_tensor("y", [T, DM], F32, kind="ExternalOutput").ap()

    with tile.TileContext(nc) as tc, ExitStack() as ctx:
        const = ctx.enter_context(tc.tile_pool(name="const", bufs=1))
        dram = ctx.enter_context(tc.tile_pool(name="dram", bufs=1, space="DRAM"))

        kv_local = dram.tile([2, H, VP, LLOC], BF16, tag="kv_local")
        kv_all = dram.tile([2, 2, H, VP, LLOC], BF16, tag="kv_all")
        xemb_d = dram.tile([DM, T], BF16, tag="xemb_d")

        # ---- constants ----
        ident = const.tile([128, 128], F32, tag="ident")
        make_identity(nc, ident)
        ones_row_f = const.tile([1, 128], F32, tag="ones_row_f")
        nc.vector.memset(ones_row_f, 1.0)
        ones_row = const.tile([1, 128], BF16, tag="ones_row")
        nc.vector.tensor_copy(ones_row, ones_row_f)
        ones_col_f = const.tile([128, 1], F32, tag="ones_col_f")
        nc.vector.memset(ones_col_f, 1.0)
        ones_col = const.tile([128, 1], BF16, tag="ones_col")
        nc.vector.tensor_copy(ones_col, ones_col_f)
        ones_blk = const.tile([128, LLOC], BF16, tag="ones_blk")
        nc.gpsimd.memset(ones_blk, 1.0)
        eps_t = const.tile([1, 1], F32, tag="eps_t")
        nc.vector.memset(eps_t, EPS)

        cvec_t = const.tile([128, NCONST], F32, tag="cvec_t")
        nc.sync.dma_start(out=cvec_t, in_=cvec_ap)

        def cslice(nm):
            off, w = _COFF[nm]
            return cvec_t[:, off:off + w]

        bv_t = cslice("bv")
        bc2_t = cslice("bc2")
        b2_t = cslice("b2")
        g1_t = cslice("g1")
        be1_t = cslice("be1")
        g2_t = cslice("g2")
        be2_t = cslice("be2")
        b1_t = cslice("b1")
        hm_t = cslice("hmask")

        bqn_t = const.tile([128, CT], F32, tag="bqn_t")
        nc.vector.tensor_scalar_mul(bqn_t, cslice("bq"), NORM)
        bkn_t = const.tile([128, CT], F32, tag="bkn_t")
        nc.vector.tensor_scalar_mul(bkn_t, cslice("bk"), NORM)

        # fused dwconv+avgpool 6-tap FIR coefficients (host-precomputed)
        gtaps = {nm: cslice("g" + nm) for nm in ("q", "k", "v")}
        dbs = {nm: cslice("db" + nm) for nm in ("q", "k", "v")}
        hcorr_t = cslice("hcorr")

        qp_ctx = ExitStack()
        qp_pool = qp_ctx.enter_context(
            tc.tile_pool(name="qp_pool", bufs=1, side="right"))
        qp_tiles = [qp_pool.tile([128, LLOC], BF16, tag=f"qp{i}", name=f"qp{i}")
                    for i in range(CT)]
        mask_tiles = [qp_pool.tile([128, LLOC], BF16, tag=f"mask{mt}",
                                   name=f"mask{mt}") for mt in range(MT)]
        kv_loads = {}

        def emit_kv_loads(hp):
            # kp rows 0-63 = head 2hp, 64-127 = head 2hp+1; vt token-major
            # via XBAR transpose with the ones row at vt[:, mt, j*VP+DD]
            kp2 = qp_pool.tile([128, 2 * LLOC], BF16, tag="kp2", bufs=3,
                               name=f"kp2_{hp}")
            for g_ in range(2):
                nc.sync.dma_start(
                    out=kp2[:, g_ * LLOC:(g_ + 1) * LLOC],
                    in_=kv_all[g_, 0, 2 * hp:2 * hp + 2, 0:DD, :])
            vt = qp_pool.tile([128, MT, 2 * VP], BF16, tag="vt", bufs=2,
                              name=f"vt{hp}")
            for mt in range(MT):
                g_, mb = mt // (MT // 2), mt % (MT // 2)
                nc.sync.dma_start(
                    out=vt[:, mt, :],
                    in_=kv_all[g_, 1, 2 * hp:2 * hp + 2, :,
                               ts(mb, 128)].rearrange("h r m -> (h r) m"),
                    transpose=True)
            kv_loads[hp] = (kp2, vt)

        owup_pool = ctx.enter_context(tc.tile_pool(name="owup_pool", bufs=1))
        owup_tiles = [
            owup_pool.tile([128, LLOC], BF16, tag=f"owup{i}", name=f"owup{i}")
            for i in range(CT)]

        proj_chunks = _chunks(TH, 512)

        # ================= Stage A+B: embed, QKV proj, dwconv, pool ========
        xe_ctx = ExitStack()
        xe_pool = xe_ctx.enter_context(tc.tile_pool(name="xe_pool", bufs=1))
        xemb_tiles = [
            xe_pool.tile([128, TH], BF16, tag=f"xemb{ci}", name=f"xemb{ci}")
            for ci in range(CT)]

        with tc.tile_pool(name="sAB", bufs=1) as sab, \
             tc.tile_pool(name="psAB", bufs=1, space="PSUM") as psab:
            xe_r = xe_ap.rearrange("(c p) t -> p c t", p=128)
            xp_r = xp_ap.rearrange("(c p) t -> p c t", p=128)
            for (c0, cw) in proj_chunks:
                xs1 = sab.tile([128, CT, 512], BF16, tag="xs1", bufs=2,
                               name=f"xs1_{c0}")
                nc.sync.dma_start(out=xs1[:, :, :cw],
                                  in_=xe_r[:, :, c0:c0 + cw])
                xs2 = sab.tile([128, CT, 512], BF16, tag="xs2", bufs=2,
                               name=f"xs2_{c0}")
                nc.scalar.dma_start(out=xs2[:, :, :cw],
                                    in_=xp_r[:, :, c0:c0 + cw])
                for ci in range(CT):
                    nc.vector.tensor_add(xemb_tiles[ci][:, c0:c0 + cw],
                                         xs1[:, ci, :cw], xs2[:, ci, :cw])

            for kind, w_ap, bias_t, scale in (
                ("k", wk_ap, bkn_t, NORM),
                ("v", wv_ap, bv_t, 1.0),
                ("q", wq_ap, bqn_t, NORM),
            ):
                wkt = sab.tile([128, CT, DM], BF16, tag="wkind", bufs=2,
                               name=f"wkind_{kind}")
                nc.sync.dma_start(out=wkt[:, 0:2, :], in_=w_ap[:, 0:2, :])
                nc.sync.dma_start(out=wkt[:, 2:CT, :], in_=w_ap[:, 2:CT, :])
                for co in range(CT):
                    wblks = [wkt[:, co, ts(ci, 128)] for ci in range(CT)]
                    pre = sab.tile([128, TH], BF16, tag="pre", bufs=3,
                                   name=f"pre{kind}{co}")
                    for (c0, cw) in proj_chunks:
                        ps = psab.tile([128, 512], F32, tag="qkv", bufs=3,
                                       name=f"ps{kind}{co}_{c0}")
                        for ci in range(CT):
                            nc.tensor.matmul(
                                ps[:, :cw], wblks[ci],
                                xemb_tiles[ci][:, c0:c0 + cw],
                                start=(ci == 0), stop=(ci == CT - 1))
                        nc.scalar.activation(pre[:, c0:c0 + cw], ps[:, :cw],
                                             AF.Identity,
                                             bias=bias_t[:, co:co + 1],
                                             scale=scale)
                    # reference zero-pads BEFORE dwconv/pooling: kill the
                    # bias-injected halo columns on first-half cores
                    nc.vector.tensor_mul(pre[:, 0:HALO], pre[:, 0:HALO], hm_t)
                    # fused causal dwconv (width 3) + avg-pool (win=4)
                    # as a 6-tap stride-4 FIR: out[l] = sum_u g_u pre[4l+3+u]
                    # + db (db corrected on column 0 of first-half cores)
                    g_t = gtaps[kind]
                    db_col = dbs[kind][:, co:co + 1]

                    def prew(u):
                        return pre[:, 3 + u:3 + u + LLOC * KER].rearrange(
                            "p (l k) -> p l k", k=KER)[:, :, 0]

                    acc = sab.tile([128, LLOC], F32, tag="acc", bufs=3,
                                   name=f"acc{kind}{co}")
                    nc.scalar.activation(acc, prew(0), AF.Identity,
                                         bias=db_col,
                                         scale=g_t[:, co * 6:co * 6 + 1])
                    for u in range(1, 5):
                        nc.vector.scalar_tensor_tensor(
                            acc, prew(u), g_t[:, co * 6 + u:co * 6 + u + 1],
                            acc, op0=AL.mult, op1=AL.add)
                    if kind == "q":
                        tgt = qp_tiles[co]
                    else:
                        tgt = sab.tile([128, LLOC], BF16, tag="kvp", bufs=2,
                                       name=f"kvp{kind}{co}")
                    nc.vector.scalar_tensor_tensor(
                        tgt, prew(5), g_t[:, co * 6 + 5:co * 6 + 6],
                        acc, op0=AL.mult, op1=AL.add)
                    nc.vector.scalar_tensor_tensor(
                        tgt[:, 0:1], db_col, hcorr_t, tgt[:, 0:1],
                        op0=AL.mult, op1=AL.add)
                    if kind != "q":
                        kvi = 0 if kind == "k" else 1
                        nc.scalar.dma_start(
                            out=kv_local[kvi, 2 * co:2 * co + 2, 0:DD, :],
                            in_=tgt)

                # single AllGather after k+v done; hides under q proj
                if kind == "v":
                    # ones row (softmax denominator) baked into v rows 64:72
                    # (and into unused k padding rows so nothing is uninit)
                    nc.scalar.dma_start(out=kv_local[0, :, DD:VP, :],
                                        in_=ones_blk)
                    nc.scalar.dma_start(out=kv_local[1, :, DD:VP, :],
                                        in_=ones_blk)
                    if mock_collective:
                        nc.gpsimd.dma_start(out=kv_all[0], in_=kv_local)
                        nc.gpsimd.dma_start(out=kv_all[1], in_=kv_local)
                    else:
                        nc.gpsimd.collective_compute(
                            "AllGather", AL.bypass,
                            replica_groups=[[0, 1], [2, 3], [4, 5], [6, 7]],
                            ins=[kv_local.opt()], outs=[kv_all.opt()])
                    emit_kv_loads(0)
                    emit_kv_loads(1)

            # spill xemb for the residual add in stage EF
            for ci in range(CT):
                nc.gpsimd.dma_start(out=xemb_d[ts(ci, 128), :],
                                    in_=xemb_tiles[ci][:, HALO:TH])
        xe_ctx.close()

        # ============ resident weights for stage EF (load during D) ========
        wres_ctx = ExitStack()
        wres = wres_ctx.enter_context(tc.tile_pool(name="wres", bufs=1))
        w2_tiles = []
        for f in range(FT):
            t_ = wres.tile([128, DM], BF16, tag=f"w2r{f}", name=f"w2r{f}")
            nc.gpsimd.dma_start(out=t_, in_=w2_ap[f])
            w2_tiles.append(t_)

        # ============ Stage D: pooled causal attention ======================
        with tc.tile_pool(name="sD", bufs=1) as sd, \
             tc.tile_pool(name="psD", bufs=1, space="PSUM") as psd:
            for mt in range(MT):
                nc.scalar.dma_start(out=mask_tiles[mt],
                                    in_=mask_ap[ts(mt, 128), :])
            for hp in range(H // 2):
                if hp not in kv_loads:
                    emit_kv_loads(hp)
                kp2, vt = kv_loads.pop(hp)

                for j in range(2):
                    h = 2 * hp + j
                    qp_h = qp_tiles[hp][j * DD:(j + 1) * DD, :]
                    kp_h = kp2[j * DD:(j + 1) * DD, :]
                    ps_av = psd.tile([DD + 1, LLOC], F32, tag="av", bufs=3,
                                     name=f"av{h}")
                    for mt in range(MT):
                        ps_lg = psd.tile([128, LLOC], F32, tag="lg", bufs=3,
                                         name=f"lg{h}_{mt}")
                        nc.tensor.matmul(ps_lg, kp_h[:, ts(mt, 128)], qp_h,
                                         start=True, stop=True,
                                         tile_position=(j * DD, 0))
                        wexp = sd.tile([128, LLOC], BF16, tag="wexp", bufs=3,
                                       name=f"wexp{h}_{mt}")
                        nc.scalar.activation(wexp, ps_lg, AF.Exp)
                        wexpm = sd.tile([128, LLOC], BF16, tag="wexpm",
                                        bufs=5, name=f"wexpm{h}_{mt}")
                        nc.vector.tensor_mul(wexpm, wexp, mask_tiles[mt])
                        nc.tensor.matmul(ps_av,
                                         vt[:, mt, j * VP:j * VP + DD + 1],
                                         wexpm,
                                         start=(mt == 0), stop=(mt == MT - 1))

                    rec = sd.tile([1, LLOC], BF16, tag="rec", bufs=2,
                                  name=f"rec{h}")
                    with nc.allow_low_precision(reason="softmax denom recip"):
                        nc.vector.reciprocal(rec, ps_av[DD:DD + 1, :])
                    ps_bc = psd.tile([DD, LLOC], F32, tag="bc", bufs=2,
                                     name=f"bc{h}")
                    nc.tensor.matmul(ps_bc, ones_row[0:1, 0:DD], rec,
                                     start=True, stop=True)
                    bc_sb = sd.tile([DD, LLOC], BF16, tag="bcs", bufs=2,
                                    name=f"bcs{h}")
                    nc.vector.tensor_copy(bc_sb, ps_bc)
                    nc.vector.tensor_mul(
                        owup_tiles[hp][j * DD:(j + 1) * DD, :],
                        ps_av[0:DD, :], bc_sb)

        qp_ctx.close()

        # ======== Stage EF: wc2+LN1 on pooled, then FFN+LN2 per chunk ======
        with tc.tile_pool(name="sEF", bufs=1) as se, \
             tc.tile_pool(name="psEF", bufs=1, space="PSUM") as pse:
            # ---- wc2 proj + LN1 on pooled tokens (once) ----
            ln1p = se.tile([128, CT, LLOC], BF16, tag="ln1p", name="ln1p")
            ps_s1 = pse.tile([1, LLOC], F32, tag="sred", bufs=2, name="ln1s1")
            ps_s2 = pse.tile([1, LLOC], F32, tag="sred", bufs=2, name="ln1s2")
            a_tiles = []
            for co in range(CT):
                wct = se.tile([128, DM], BF16, tag="wcs", bufs=2,
                              name=f"wcs{co}")
                nc.scalar.dma_start(out=wct, in_=wc2_ap[co])
                ps_wc = pse.tile([128, LLOC], F32, tag="mm1", bufs=2,
                                 name=f"pswc{co}")
                wcb = [wct[:, ts(ci, 128)] for ci in range(CT)]
                for ci in range(CT):
                    nc.tensor.matmul(ps_wc, wcb[ci], owup_tiles[ci],
                                     start=(ci == 0), stop=(ci == CT - 1))
                a_sb = se.tile([128, LLOC], BF16, tag="asb", bufs=CT,
                               name=f"asb{co}")
                nc.scalar.activation(a_sb, ps_wc, AF.Identity,
                                     bias=bc2_t[:, co:co + 1], scale=1.0)
                a2 = se.tile([128, LLOC], BF16, tag="a2", bufs=1,
                             name=f"a2_{co}")
                nc.vector.tensor_mul(a2, a_sb, a_sb)
                nc.tensor.matmul(ps_s1, ones_col, a_sb,
                                 start=(co == 0), stop=(co == CT - 1))
                nc.tensor.matmul(ps_s2, ones_col, a2,
                                 start=(co == 0), stop=(co == CT - 1))
                a_tiles.append(a_sb)

            mean = se.tile([1, LLOC], BF16, tag="mean", bufs=1, name="mean1")
            nc.vector.tensor_scalar_mul(mean, ps_s1, 1.0 / DM)
            e2 = se.tile([1, LLOC], F32, tag="e2", bufs=1, name="e2_1")
            nc.vector.tensor_scalar_mul(e2, ps_s2, 1.0 / DM)
            m2 = se.tile([1, LLOC], F32, tag="m2", bufs=1, name="m2_1")
            nc.vector.tensor_mul(m2, mean, mean)
            var = se.tile([1, LLOC], F32, tag="var", bufs=1, name="var1")
            nc.vector.tensor_sub(var, e2, m2)
            sd_t = se.tile([1, LLOC], F32, tag="sd", bufs=1, name="sd1")
            nc.scalar.activation(sd_t, var, AF.Sqrt, bias=eps_t[0:1, 0:1])
            rstd = se.tile([1, LLOC], BF16, tag="rstd", bufs=1, name="rstd1")
            with nc.allow_low_precision(reason="bf16 rstd"):
                nc.vector.reciprocal(rstd, sd_t)

            ps_mb = pse.tile([128, LLOC], F32, tag="bcps", bufs=2, name="mb1")
            nc.tensor.matmul(ps_mb, ones_row, mean, start=True, stop=True)
            mbc = se.tile([128, LLOC], BF16, tag="mbc", bufs=1, name="mbc1")
            nc.vector.tensor_copy(mbc, ps_mb)
            ps_rb = pse.tile([128, LLOC], F32, tag="bcps", bufs=2, name="rb1")
            nc.tensor.matmul(ps_rb, ones_row, rstd, start=True, stop=True)
            rbc = se.tile([128, LLOC], BF16, tag="rbc", bufs=1, name="rbc1")
            nc.vector.tensor_copy(rbc, ps_rb)

            for co in range(CT):
                v1 = se.tile([128, LLOC], BF16, tag="lnv", bufs=1,
                             name=f"lnv{co}")
                nc.vector.tensor_sub(v1, a_tiles[co], mbc)
                v2 = se.tile([128, LLOC], BF16, tag="lnu", bufs=1,
                             name=f"lnu{co}")
                nc.gpsimd.tensor_mul(v2, v1, rbc)
                nc.vector.tensor_scalar(ln1p[:, co, :], v2,
                                        g1_t[:, co:co + 1],
                                        be1_t[:, co:co + 1],
                                        op0=AL.mult, op1=AL.add)

            # ---- per 512-token chunk: residual+upsample, FFN, LN2, out ----
            xemb_r = xemb_d.rearrange("(c p) t -> p c t", p=128)
            for (c0, cw) in _chunks(T, 512):
                p0 = c0 // KER
                x1c = se.tile([128, CT, 512], BF16, tag="x1c", bufs=2,
                              name=f"x1c{c0}")
                nc.sync.dma_start(out=x1c, in_=xemb_r[:, :, c0:c0 + cw])
                for co in range(CT):
                    x13 = x1c[:, co, :].rearrange("p (l r) -> p l r", r=KER)
                    nc.vector.tensor_add(
                        x13, x13,
                        ln1p[:, co, p0:p0 + 128].unsqueeze(2)
                        .broadcast_to((128, 128, KER)))

                hb_tiles = []
                w1g = None
                for f in range(FT):
                    if f % 4 == 0:
                        w1g = se.tile([128, 4, DM], BF16, tag="w1s", bufs=2,
                                      name=f"w1g{f}_{c0}")
                        nc.sync.dma_start(out=w1g,
                                          in_=w1_ap[:, f:f + 4, :])
                    ps_h = pse.tile([128, 512], F32, tag="mm1", bufs=2,
                                    name=f"psh{f}_{c0}")
                    for ci in range(CT):
                        nc.tensor.matmul(ps_h, w1g[:, f % 4, ts(ci, 128)],
                                         x1c[:, ci, :],
                                         start=(ci == 0), stop=(ci == CT - 1))
                    hr = se.tile([128, 512], BF16, tag="hr", bufs=2,
                                 name=f"hr{f}_{c0}")
                    nc.scalar.activation(hr, ps_h, AF.Relu,
                                         bias=b1_t[:, f:f + 1], scale=1.0)
                    hb = se.tile([128, 512], BF16, tag=f"hb{f}",
                                 name=f"hb{f}_{c0}")
                    nc.vector.tensor_mul(hb, hr, hr)
                    hb_tiles.append(hb)

                ps_t1 = pse.tile([1, 512], F32, tag="sred", bufs=2,
                                 name=f"fs1_{c0}")
                ps_t2 = pse.tile([1, 512], F32, tag="sred", bufs=2,
                                 name=f"fs2_{c0}")
                ffw_tiles = []
                for co in range(CT):
                    ps_y = pse.tile([128, 512], F32, tag="yps", bufs=2,
                                    name=f"psy{co}_{c0}")
                    for f in range(FT):
                        nc.tensor.matmul(ps_y, w2_tiles[f][:, ts(co, 128)],
                                         hb_tiles[f],
                                         start=(f == 0), stop=(f == FT - 1))
                    ffw = se.tile([128, 512], BF16, tag="fsb", bufs=CT,
                                  name=f"ffw{co}_{c0}")
                    nc.scalar.activation(ffw, ps_y, AF.Identity,
                                         bias=b2_t[:, co:co + 1], scale=1.0)
                    f2 = se.tile([128, 512], BF16, tag="f2", bufs=1,
                                 name=f"f2_{co}_{c0}")
                    nc.vector.tensor_mul(f2, ffw, ffw)
                    nc.tensor.matmul(ps_t1, ones_col, ffw,
                                     start=(co == 0), stop=(co == CT - 1))
                    nc.tensor.matmul(ps_t2, ones_col, f2,
                                     start=(co == 0), stop=(co == CT - 1))
                    ffw_tiles.append(ffw)

                fmean = se.tile([1, 512], BF16, tag="fmean", bufs=1,
                                name=f"fmean{c0}")
                nc.vector.tensor_scalar_mul(fmean, ps_t1, 1.0 / DM)
                fe2 = se.tile([1, 512], F32, tag="fe2", bufs=1,
                              name=f"fe2_{c0}")
                nc.vector.tensor_scalar_mul(fe2, ps_t2, 1.0 / DM)
                fm2 = se.tile([1, 512], F32, tag="fm2", bufs=1,
                              name=f"fm2_{c0}")
                nc.vector.tensor_mul(fm2, fmean, fmean)
                fvar = se.tile([1, 512], F32, tag="fvar", bufs=1,
                               name=f"fvar{c0}")
                nc.vector.tensor_sub(fvar, fe2, fm2)
                fsd = se.tile([1, 512], F32, tag="fsd", bufs=1,
                              name=f"fsd{c0}")
                nc.scalar.activation(fsd, fvar, AF.Sqrt, bias=eps_t[0:1, 0:1])
                frstd = se.tile([1, 512], BF16, tag="frstd", bufs=1,
                                name=f"frstd{c0}")
                with nc.allow_low_precision(reason="bf16 rstd"):
                    nc.vector.reciprocal(frstd, fsd)

                ps_fmb = pse.tile([128, 512], F32, tag="bcps", bufs=2,
                                  name=f"fmb{c0}")
                nc.tensor.matmul(ps_fmb, ones_row, fmean, start=True,
                                 stop=True)
                fmbc = se.tile([128, 512], BF16, tag="fmbc", bufs=1,
                               name=f"fmbc{c0}")
                nc.vector.tensor_copy(fmbc, ps_fmb)
                ps_frb = pse.tile([128, 512], F32, tag="bcps", bufs=2,
                                  name=f"frb{c0}")
                nc.tensor.matmul(ps_frb, ones_row, frstd, start=True,
                                 stop=True)
                frbc = se.tile([128, 512], BF16, tag="frbc", bufs=1,
                               name=f"frbc{c0}")
                nc.vector.tensor_copy(frbc, ps_frb)

                for co in range(CT):
                    v1 = se.tile([128, 512], BF16, tag="flnv", bufs=2,
                                 name=f"flnv{co}_{c0}")
                    nc.vector.tensor_sub(v1, ffw_tiles[co], fmbc)
                    v2 = se.tile([128, 512], BF16, tag="flnu", bufs=2,
                                 name=f"flnu{co}_{c0}")
                    nc.gpsimd.tensor_mul(v2, v1, frbc)
                    yfm = se.tile([128, 512], F32, tag="yfm", bufs=2,
                                  name=f"yfm{co}_{c0}")
                    nc.vector.tensor_scalar(yfm, v2, g2_t[:, co:co + 1],
                                            be2_t[:, co:co + 1],
                                            op0=AL.mult, op1=AL.add)
                    nc.vector.tensor_add(yfm, yfm, x1c[:, co, :])
                    # transpose to token-major; one batched DMA per co
                    ytb = se.tile([128, 4, 128], F32, tag="ytb", bufs=1,
                                  name=f"ytb{co}_{c0}")
                    for tb in range(4):
                        ps_t = pse.tile([128, 128], F32, tag="bcps", bufs=2,
                                        name=f"ytr{co}_{tb}_{c0}")
                        nc.tensor.transpose(ps_t, yfm[:, ts(tb, 128)], ident)
                        nc.vector.tensor_copy(ytb[:, tb, :], ps_t)
                    nc.scalar.dma_start(
                        out=y_ap[c0:c0 + 512, ts(co, 128)].rearrange(
                            "(tb p) c -> p tb c", p=128),
                        in_=ytb)
        wres_ctx.close()

    nc.compile()
    return nc


_PROGRAM_CACHE = {}


def _get_program(S=S_FULL):
    if S not in _PROGRAM_CACHE:
        _PROGRAM_CACHE[S] = build_program(S)
    return _PROGRAM_CACHE[S]


def _vec_fold(v, cols):
    """[N] -> [128, N//128] with column i = v[i*128:(i+1)*128]."""
    v = np.asarray(v, np.float32)
    return np.ascontiguousarray(v.reshape(cols, 128).T)


def _bf(x):
    return np.ascontiguousarray(np.asarray(x, np.float32)
                                .astype(ml_dtypes.bfloat16))


def prep_inputs(inputs, S=S_FULL):
    T = S // 2
    L = S // KER
    LLOC = L // 2

    g = {k: np.asarray(v, np.float32) for k, v in inputs.items()}

    def wtile(w, nt):
        ci = w.shape[0] // 128
        return np.ascontiguousarray(
            w.reshape(ci, 128, nt, 128).transpose(2, 1, 0, 3)
            .reshape(nt, 128, ci * 128))

    # fold wup (+bup) into wc: attention output is constant within each
    # 4-token group, so wc2 runs on pooled tokens
    wc2 = np.kron(np.eye(H, dtype=np.float32), g["wup"]) @ g["wc"]
    bc2 = np.tile(g["bup"], H) @ g["wc"] + g["bc"]

    def wkind(w):
        # [128, CT(co), DM(ci*128+j)]: per-partition-contiguous for 1 DMA
        return np.ascontiguousarray(
            w.reshape(CT, 128, CT, 128).transpose(1, 2, 0, 3)
            .reshape(128, CT, DM))

    shared = {
        "wq": _bf(wkind(g["wq"])), "wk": _bf(wkind(g["wk"])),
        "wv": _bf(wkind(g["wv"])), "wc2": _bf(wtile(wc2, CT)),
        "w1": _bf(np.ascontiguousarray(
            g["w1"].reshape(CT, 128, FT, 128).transpose(1, 2, 0, 3)
            .reshape(128, FT, DM))),
        "w2": _bf(g["w2"].reshape(FT, 128, DM)),
    }
    cvec = np.zeros((128, NCONST), np.float32)

    def setc(nm, arr):
        off, w = _COFF[nm]
        assert arr.shape == (128, w), (nm, arr.shape)
        cvec[:, off:off + w] = arr

    for nm in ("bq", "bk", "bv", "b2", "g1", "be1", "g2", "be2",
               "dbq", "dbk", "dbv"):
        setc(nm, _vec_fold(g[nm], CT))
    setc("bc2", _vec_fold(bc2, CT))
    setc("b1", _vec_fold(g["b1"], FT))
    for nm in ("q", "k", "v"):
        t = g["dw" + nm]  # [3, DM] causal taps
        gf = np.stack([t[0], t[0] + t[1], t[0] + t[1] + t[2],
                       t[0] + t[1] + t[2], t[1] + t[2], t[2]]) / KER  # [6,DM]
        setc("g" + nm, gf.T.reshape(CT, 128, 6).transpose(1, 0, 2)
             .reshape(128, CT * 6))

    in_maps = []
    for c in range(N_CORES):
        b, hf = c // 2, c % 2
        m = dict(shared)
        for nm, arr in (("xe", g["x_enc"]), ("xp", g["x_pos"])):
            fm = arr[b].T  # [DM, S]
            if hf == 0:
                sl = np.concatenate(
                    [np.zeros((DM, HALO), np.float32), fm[:, :T]], axis=1)
            else:
                sl = fm[:, T - HALO:2 * T]
            m[nm] = _bf(sl)
        cv = cvec.copy()
        cv[:, _COFF["hmask"][0]:_COFF["hmask"][0] + HALO] = float(hf)
        cv[:, _COFF["hcorr"][0]] = -0.75 if hf == 0 else 0.0
        m["cvec"] = cv
        m["mask"] = _bf(
            (np.arange(L)[:, None] <= (hf * LLOC + np.arange(LLOC))[None, :])
            .astype(np.float32))
        in_maps.append(m)
    return in_maps


def gather_output(results, S=S_FULL):
    T = S // 2
    y = np.empty((B, S, DM), np.float32)
    for c in range(N_CORES):
        b, hf = c // 2, c % 2
        y[b, hf * T:(hf + 1) * T, :] = results[c]["y"]
    return y


def kernel(**inputs):
    nc = _get_program(S_FULL)
    in_maps = prep_inputs(inputs, S_FULL)
    res = run_bass_kernel_spmd(nc, in_maps, list(range(N_CORES)))
    return gather_output(res.results, S_FULL)
